# revision 4
# baseline (speedup 1.0000x reference)
"""Trainium2 Bass kernel for pairwise Jaccard similarity (nn_ConceptSpace).

Math (per the reference):
    a1 = sigmoid(x1)  [1024, 256]
    a2 = sigmoid(x2)  [1024, 256]
    inter[i, j] = sum_d min(a1[i, d], a2[j, d])
    union[i, j] = s1[i] + s2[j] - inter[i, j]
    out = (sim, sim.T) with sim = inter / union

Sharding: x1 rows are split across the 8 cores (128 rows each); x2 is
replicated.  Each core computes its [128, 1024] slice of `sim`.  sim.T is a
free host-side transpose after gathering.

Per-core device algorithm (layout: d on partitions):
  - a2T tiles [128 d, 1024 j] (bf16, via ACT sigmoid), for d-halves dt=0,1
  - a1T tile  [128 d, 256]    (fp32) holding sigmoid(x1 slice) transposed
  - for each of the 128 local rows i:
      m = tensor_scalar_min(a2T[dt][:, jb], a1T[:, dt*128+i])   (DVE, 4x bf16)
      psum[jb] += onehot_i.T @ m                                 (PE matmul)
    The one-hot stationary matrix selects output partition i, so PSUM
    accumulates the full [128 i, 512 j] `inter` block per j-half.
  - epilogue: union = (s1 + s2) - inter;  sim = inter * recip(union)
"""

import sys
from contextlib import ExitStack

for _p in ("/opt/trn_rl_repo", "/root/.axon_site", "/root/.axon_site/_ro/trn_rl_repo",
           "/root/.axon_site/_ro/pypackages"):
    if _p not in sys.path:
        sys.path.insert(0, _p)

import numpy as np

N = 1024          # rows of x1 / x2
D = 256           # feature dim
NCORES = 8
RP = N // NCORES  # rows per core = 128
P = 128           # partitions
JB = 512          # j-block (one PSUM bank of fp32)
NJB = N // JB     # 2 j-blocks


def _build_program():
    import concourse.bass as bass
    import concourse.tile as tile
    from concourse import bacc, mybir

    f32 = mybir.dt.float32
    bf16 = mybir.dt.bfloat16
    AF = mybir.ActivationFunctionType
    Alu = mybir.AluOpType

    nc = bacc.Bacc(trn_type="TRN2", debug=False, target_bir_lowering=False)

    x1t = nc.dram_tensor("x1t", [D, RP], f32, kind="ExternalInput")    # x1 slice, transposed
    x1n = nc.dram_tensor("x1n", [RP, D], f32, kind="ExternalInput")    # x1 slice, natural
    x2t = nc.dram_tensor("x2t", [D, N], f32, kind="ExternalInput")     # x2 full, transposed
    simo = nc.dram_tensor("simo", [RP, N], f32, kind="ExternalOutput")

    with ExitStack() as ctx:
        tc = ctx.enter_context(tile.TileContext(nc))
        const = ctx.enter_context(tc.tile_pool(name="const", bufs=1))
        minp = ctx.enter_context(tc.tile_pool(name="minp", bufs=8))
        finp = ctx.enter_context(tc.tile_pool(name="finp", bufs=2))
        psum = ctx.enter_context(
            tc.tile_pool(name="psum", bufs=1, space=bass.MemorySpace.PSUM)
        )

        # ---- load + sigmoid ------------------------------------------------
        X2T = [const.tile([P, N], f32, tag=f"x2t{dt}", name=f"x2t{dt}") for dt in range(2)]
        A2 = [const.tile([P, N], bf16, tag=f"a2{dt}", name=f"a2{dt}") for dt in range(2)]
        for dt in range(2):
            nc.sync.dma_start(X2T[dt][:], x2t[dt * P:(dt + 1) * P, :])
            nc.scalar.activation(A2[dt][:], X2T[dt][:], AF.Sigmoid)

        X1T = const.tile([P, D], f32, tag="x1t", name="x1ts")   # [d_low, (dt, i)] halves
        A1 = const.tile([P, D], f32, tag="a1", name="a1")
        for dt in range(2):
            hs = slice(dt * RP, (dt + 1) * RP)
            nc.sync.dma_start(X1T[:, hs], x1t[dt * P:(dt + 1) * P, :])
            nc.scalar.activation(A1[:, hs], X1T[:, hs], AF.Sigmoid)

        # s1[i] = sum_d sigmoid(x1[i, d])  (fp32, i on partitions)
        X1N = const.tile([RP, D], f32, tag="x1n", name="x1ns")
        nc.sync.dma_start(X1N[:], x1n[:])
        scr1 = const.tile([RP, D], bf16, tag="scr1", name="scr1")
        s1 = const.tile([RP, 1], f32, tag="s1", name="s1")
        nc.scalar.activation(scr1[:], X1N[:], AF.Sigmoid, accum_out=s1[:])

        # ---- constants -----------------------------------------------------
        # Sliding one-hot buffer: col 127 is ones, everything else zero.
        # W_i = B[:, 127-i : 255-i] is the one-hot matrix with ones in col i.
        B = const.tile([P, 2 * P - 1], bf16, tag="onehot", name="onehot")
        nc.gpsimd.memset(B[:], 0.0)
        nc.gpsimd.memset(B[:, P - 1:P], 1.0)
        onescol = const.tile([P, 1], bf16, tag="onescol", name="onescol")
        nc.gpsimd.memset(onescol[:], 1.0)
        onesrow = const.tile([1, P], f32, tag="onesrow", name="onesrow")
        nc.gpsimd.memset(onesrow[:], 1.0)

        # ---- s2 (row sums of a2) and Sb = s1[i] + s2[j] tiles --------------
        s2row = const.tile([1, N], f32, tag="s2row", name="s2row")
        Sb = [const.tile([P, JB], f32, tag=f"sb{jb}", name=f"sb{jb}") for jb in range(NJB)]
        for jb in range(NJB):
            js = slice(jb * JB, (jb + 1) * JB)
            s2p = psum.tile([1, JB], f32, tag="s2p", name="s2p")
            for dt in range(2):
                nc.tensor.matmul(
                    s2p[:], onescol[:], A2[dt][:, js],
                    start=(dt == 0), stop=(dt == 1),
                )
            nc.scalar.copy(s2row[:, js], s2p[:])
            sbp = psum.tile([P, JB], f32, tag="sbp", name="sbp")
            nc.tensor.matmul(sbp[:], onesrow[:], s2row[:, js], start=True, stop=True)
            nc.vector.tensor_scalar_add(Sb[jb][:], sbp[:], s1[:])

        # ---- main loop: inter = sum_d min(a1, a2) --------------------------
        acc = [psum.tile([P, JB], f32, tag=f"acc{jb}", name=f"acc{jb}") for jb in range(NJB)]
        for i in range(RP):
            w = B[:, P - 1 - i:2 * P - 1 - i]
            for jb in range(NJB):
                js = slice(jb * JB, (jb + 1) * JB)
                for dt in range(2):
                    m = minp.tile([P, JB], bf16, tag="m", name="m")
                    nc.vector.tensor_scalar_min(
                        m[:], A2[dt][:, js], A1[:, dt * RP + i:dt * RP + i + 1]
                    )
                    nc.tensor.matmul(
                        acc[jb][:], w, m[:],
                        start=(i == 0 and dt == 0),
                        stop=(i == RP - 1 and dt == 1),
                    )

        # ---- epilogue: sim = inter / (Sb - inter) --------------------------
        for jb in range(NJB):
            js = slice(jb * JB, (jb + 1) * JB)
            interf = finp.tile([P, JB], f32, tag="interf", name="interf")
            nc.scalar.copy(interf[:], acc[jb][:])
            union = finp.tile([P, JB], f32, tag="union", name="union")
            nc.vector.tensor_sub(union[:], Sb[jb][:], interf[:])
            rcp = finp.tile([P, JB], f32, tag="rcp", name="rcp")
            nc.vector.reciprocal_approx_fast(rcp[:], union[:])
            sims = finp.tile([P, JB], f32, tag="sims", name="sims")
            nc.vector.tensor_mul(sims[:], interf[:], rcp[:])
            nc.sync.dma_start(simo[:, js], sims[:])

    nc.compile()
    return nc


_PROGRAM = None


def _get_program():
    global _PROGRAM
    if _PROGRAM is None:
        _PROGRAM = _build_program()
    return _PROGRAM


def _make_in_maps(x1, x2):
    x2t = np.ascontiguousarray(x2.T)
    in_maps = []
    for c in range(NCORES):
        sl = slice(c * RP, (c + 1) * RP)
        in_maps.append({
            "x1t": np.ascontiguousarray(x1[sl].T),
            "x1n": np.ascontiguousarray(x1[sl]),
            "x2t": x2t,
        })
    return in_maps


def kernel(x1, x2):
    x1 = np.asarray(x1, dtype=np.float32)
    x2 = np.asarray(x2, dtype=np.float32)
    from concourse.bass_utils import run_bass_kernel_spmd

    nc = _get_program()
    res = run_bass_kernel_spmd(nc, _make_in_maps(x1, x2), core_ids=list(range(NCORES)))
    sim = np.concatenate([res.results[c]["simo"] for c in range(NCORES)], axis=0)
    return (sim, np.ascontiguousarray(sim.T))


# revision 12
# speedup vs baseline: 1.0372x; 1.0372x over previous
"""Trainium2 Bass kernel for pairwise Jaccard similarity (nn_ConceptSpace).

Math (per the reference):
    a1 = sigmoid(x1)  [1024, 256]
    a2 = sigmoid(x2)  [1024, 256]
    inter[i, j] = sum_d min(a1[i, d], a2[j, d])
    union[i, j] = s1[i] + s2[j] - inter[i, j]
    out = (sim, sim.T) with sim = inter / union

Sharding: x1 rows are split across the 8 cores (128 rows each); x2 is
replicated.  Each core computes its [128, 1024] slice of `sim`; sim.T is a
free host-side transpose after gathering.

Per-core device algorithm (layout: d on partitions, j on free):
  - a2T units [128 d, 1024 j] (bf16) for d-halves dt=0,1; a1T [128 d, 256] fp32.
  - The [128 i, 1024 j] `inter` block accumulates in PSUM via PE matmuls whose
    stationary operand is a sliding one-hot (column i), reducing each pairwise
    tile over d (partitions) straight into output row i.
  - Pairwise tiles for row i are produced three ways to balance engines:
      * DVE tensor_scalar_min(a2T[dt], a1T[:, i])           -> min tile (4x bf16)
      * ACT activation(Abs, a2T[1], scale=.5, bias=-.5*a1)  -> 0.5|a2-a1| tile,
        accumulated with a NEGATIVE one-hot; the missing 0.5*(s1'+s2') is added
        back into PSUM with two rank-1 matmuls (mask vector x row vector).
      * a fraction of rows folds min0 - h1 on DVE (one TT op) to halve the
        PE streaming cost for that row.
  - epilogue: union = (s1 + s2) - inter;  sim = inter * recip(union)
"""

import sys
from contextlib import ExitStack

for _p in ("/opt/trn_rl_repo", "/root/.axon_site", "/root/.axon_site/_ro/trn_rl_repo",
           "/root/.axon_site/_ro/pypackages"):
    if _p not in sys.path:
        sys.path.insert(0, _p)

import numpy as np

N = 1024          # rows of x1 / x2
D = 256           # feature dim
NCORES = 8
RP = N // NCORES  # rows per core = 128
P = 128           # partitions
JB = 512          # j-block (one PSUM bank of fp32)
NJB = N // JB     # 2 j-blocks

# Row-class split (engine balance):
#   rows [0, ACT_N)              : ACT absdiff + DVE min (+ DVE fold on a
#                                  Bresenham-spread subset of FOLD_N rows)
#   rows [ACT_N, ACT_N+POOL_N)   : DVE min (dt0) + GPSIMD min (dt1)
#   rows [ACT_N+POOL_N, RP)      : DVE min x2
FOLD_N = 52
ACT_N = 70
POOL_N = 54


def _build_program(fold_n=FOLD_N, act_n=ACT_N, pool_n=POOL_N, min_bufs=10, h_bufs=8):
    import concourse.bass as bass
    import concourse.tile as tile
    from concourse import bacc, mybir

    f32 = mybir.dt.float32
    bf16 = mybir.dt.bfloat16
    AF = mybir.ActivationFunctionType

    nc = bacc.Bacc(trn_type="TRN2", debug=False, target_bir_lowering=False)

    x1t = nc.dram_tensor("x1t", [D, RP], f32, kind="ExternalInput")    # x1 slice, transposed
    x1n = nc.dram_tensor("x1n", [RP, D], f32, kind="ExternalInput")    # x1 slice, natural
    x2t = nc.dram_tensor("x2t", [D, N], f32, kind="ExternalInput")     # x2 full, transposed
    simo = nc.dram_tensor("simo", [RP, N], f32, kind="ExternalOutput")

    with ExitStack() as ctx:
        tc = ctx.enter_context(tile.TileContext(nc))
        const = ctx.enter_context(tc.tile_pool(name="const", bufs=1))
        minp = ctx.enter_context(tc.tile_pool(name="minp", bufs=min_bufs))
        hp = ctx.enter_context(tc.tile_pool(name="hp", bufs=h_bufs))
        finp = ctx.enter_context(tc.tile_pool(name="finp", bufs=2))
        psum = ctx.enter_context(
            tc.tile_pool(name="psum", bufs=1, space=bass.MemorySpace.PSUM)
        )

        # ---- load + sigmoid ------------------------------------------------
        X2T = [const.tile([P, N], f32, tag=f"x2t{dt}", name=f"x2t{dt}") for dt in range(2)]
        A2 = [const.tile([P, N], bf16, tag=f"a2{dt}", name=f"a2{dt}") for dt in range(2)]
        for dt in range(2):
            nc.sync.dma_start(X2T[dt][:], x2t[dt * P:(dt + 1) * P, :])
            nc.scalar.activation(A2[dt][:], X2T[dt][:], AF.Sigmoid)

        X1T = const.tile([P, D], f32, tag="x1t", name="x1ts")   # [d_low, (dt, i)]
        A1 = const.tile([P, D], f32, tag="a1", name="a1")
        for dt in range(2):
            hs = slice(dt * RP, (dt + 1) * RP)
            nc.sync.dma_start(X1T[:, hs], x1t[dt * P:(dt + 1) * P, :])
            nc.scalar.activation(A1[:, hs], X1T[:, hs], AF.Sigmoid)
        # negated/halved a1 (dt=1 half) as per-partition bias for ACT absdiff
        A1N = const.tile([P, RP], f32, tag="a1n", name="a1n")
        nc.vector.tensor_scalar_mul(A1N[:], A1[:, RP:D], -0.5)
        # bf16 copy of a1 dt=1 half (for the s1' partition-sum matmul)
        A1B = const.tile([P, RP], bf16, tag="a1b", name="a1b")
        nc.vector.tensor_copy(A1B[:], A1[:, RP:D])

        # s1[i] = sum_d sigmoid(x1[i, d])  (fp32, i on partitions)
        X1N = const.tile([RP, D], f32, tag="x1n", name="x1ns")
        nc.sync.dma_start(X1N[:], x1n[:])
        scr1 = const.tile([RP, D], bf16, tag="scr1", name="scr1")
        s1 = const.tile([RP, 1], f32, tag="s1", name="s1")
        nc.scalar.activation(scr1[:], X1N[:], AF.Sigmoid, accum_out=s1[:])

        # ---- constants -----------------------------------------------------
        # Sliding one-hot buffers: col P-1 is +-1, everything else zero.
        # Bp[:, P-1-i : 2P-1-i] is the one-hot matrix with +1 in col i.
        Bp = const.tile([P, 2 * P - 1], bf16, tag="onehotp", name="onehotp")
        nc.gpsimd.memset(Bp[:], 0.0)
        nc.gpsimd.memset(Bp[:, P - 1:P], 1.0)
        Bn = const.tile([P, 2 * P - 1], bf16, tag="onehotn", name="onehotn")
        nc.gpsimd.memset(Bn[:], 0.0)
        nc.gpsimd.memset(Bn[:, P - 1:P], -1.0)
        onescol = const.tile([P, 1], bf16, tag="onescol", name="onescol")
        nc.gpsimd.memset(onescol[:], 1.0)
        onesrow = const.tile([1, P], f32, tag="onesrow", name="onesrow")
        nc.gpsimd.memset(onesrow[:], 1.0)
        ones512 = const.tile([1, JB], f32, tag="ones512", name="ones512")
        nc.gpsimd.memset(ones512[:], 1.0)
        # ACT-row mask vectors: rows [0, act_n) use the absdiff path
        vmaskf = const.tile([1, P], f32, tag="vmaskf", name="vmaskf")
        nc.gpsimd.memset(vmaskf[:], 0.0)
        nc.gpsimd.memset(vmaskf[:, 0:act_n], 1.0)

        # ---- s2 rows, Sb = s1 + s2 tiles, correction vectors ---------------
        s2row = const.tile([1, N], f32, tag="s2row", name="s2row")
        s2hrow = const.tile([1, N], f32, tag="s2hrow", name="s2hrow")  # 0.5*s2'(dt1)
        Sb = [const.tile([P, JB], f32, tag=f"sb{jb}", name=f"sb{jb}") for jb in range(NJB)]
        for jb in range(NJB):
            js = slice(jb * JB, (jb + 1) * JB)
            s2p = psum.tile([1, JB], f32, tag="s2p", name="s2p")
            for dt in range(2):
                nc.tensor.matmul(
                    s2p[:], onescol[:], A2[dt][:, js],
                    start=(dt == 0), stop=(dt == 1),
                )
            nc.vector.tensor_copy(s2row[:, js], s2p[:])
            s2hp = psum.tile([1, JB], f32, tag="s2hp", name="s2hp")
            nc.tensor.matmul(s2hp[:], onescol[:], A2[1][:, js], start=True, stop=True)
            nc.vector.tensor_scalar_mul(s2hrow[:, js], s2hp[:], 0.5)
            sbp = psum.tile([P, JB], f32, tag="sbp", name="sbp")
            nc.tensor.matmul(sbp[:], onesrow[:], s2row[:, js], start=True, stop=True)
            nc.vector.tensor_scalar_add(Sb[jb][:], sbp[:], s1[:])

        # w1[m] = mask[m] * 0.5 * s1'[m]  (s1' = dt1-half row sums of a1)
        s1hp = psum.tile([1, P], f32, tag="s1hp", name="s1hp")
        nc.tensor.matmul(s1hp[:], onescol[:], A1B[:], start=True, stop=True)
        w1f = const.tile([1, P], f32, tag="w1f", name="w1f")
        nc.vector.tensor_scalar_mul(w1f[:], s1hp[:], 0.5)
        w1 = const.tile([1, P], f32, tag="w1", name="w1")
        nc.vector.tensor_mul(w1[:], w1f[:], vmaskf[:])

        # ---- main loop: accumulate inter rows into PSUM --------------------
        acc = [psum.tile([P, JB], f32, tag=f"acc{jb}", name=f"acc{jb}") for jb in range(NJB)]

        def mm(unit, w, i, jb, first, last):
            nc.tensor.matmul(
                acc[jb][:], w, unit[:, jb * JB:(jb + 1) * JB],
                start=first, stop=last,
            )

        # Emission order interleaves ACT rows (indices [0, act_n), which the
        # mask vectors rely on being contiguous) among the DVE rows, so the
        # in-order PE stream never waits on a burst of slow ACT producers.
        # Fold rows are Bresenham-spread across the ACT rows.
        counts = {"a": act_n, "p": pool_n, "d": RP - act_n - pool_n}
        base = {"a": 0, "p": act_n, "d": act_n + pool_n}
        emitted = {"a": 0, "p": 0, "d": 0}
        order = []
        for e in range(RP):
            c = max(counts, key=lambda k: counts[k] * (e + 1) / RP - emitted[k])
            order.append(base[c] + emitted[c])
            emitted[c] += 1
        assert sorted(order) == list(range(RP))

        for step, i in enumerate(order):
            wp = Bp[:, P - 1 - i:2 * P - 1 - i]
            wn = Bn[:, P - 1 - i:2 * P - 1 - i]
            use_act = i < act_n
            use_fold = use_act and act_n > 0 and (
                (i * fold_n) // act_n != ((i + 1) * fold_n) // act_n
            )
            first = step == 0

            m0 = minp.tile([P, N], bf16, tag="m", name="m")
            nc.vector.tensor_scalar_min(m0[:], A2[0][:], A1[:, i:i + 1])
            if use_act:
                h1 = hp.tile([P, N], bf16, tag="h", name="h")
                nc.scalar.activation(
                    h1[:], A2[1][:], AF.Abs, bias=A1N[:, i:i + 1], scale=0.5
                )
                if use_fold:
                    fu = minp.tile([P, N], bf16, tag="m", name="fold")
                    nc.vector.tensor_sub(fu[:], m0[:], h1[:])
                    for jb in range(NJB):
                        mm(fu, wp, i, jb, first, False)
                else:
                    for jb in range(NJB):
                        mm(m0, wp, i, jb, first, False)
                        mm(h1, wn, i, jb, False, False)
            else:
                m1 = minp.tile([P, N], bf16, tag="m", name="m1")
                if i < act_n + pool_n:
                    nc.gpsimd.tensor_scalar_min(m1[:], A2[1][:], A1[:, RP + i:RP + i + 1])
                else:
                    nc.vector.tensor_scalar_min(m1[:], A2[1][:], A1[:, RP + i:RP + i + 1])
                for jb in range(NJB):
                    mm(m0, wp, i, jb, first, False)
                    mm(m1, wp, i, jb, False, False)

        # rank-1 corrections for ACT rows: + mask[m]*0.5*s2'[j] + w1[m]*1
        for jb in range(NJB):
            js = slice(jb * JB, (jb + 1) * JB)
            nc.tensor.matmul(acc[jb][:], vmaskf[:], s2hrow[:, js], start=False, stop=False)
            nc.tensor.matmul(acc[jb][:], w1[:], ones512[:], start=False, stop=True)

        # ---- epilogue: sim = inter / (Sb - inter) --------------------------
        for jb in range(NJB):
            js = slice(jb * JB, (jb + 1) * JB)
            union = finp.tile([P, JB], f32, tag="union", name="union")
            nc.vector.tensor_sub(union[:], Sb[jb][:], acc[jb][:])
            rcp = finp.tile([P, JB], f32, tag="rcp", name="rcp")
            nc.vector.reciprocal_approx_fast(rcp[:], union[:])
            sims = finp.tile([P, JB], f32, tag="sims", name="sims")
            nc.vector.tensor_mul(sims[:], acc[jb][:], rcp[:])
            nc.sync.dma_start(simo[:, js], sims[:])

    nc.compile()
    return nc


_PROGRAM = None


def _get_program():
    global _PROGRAM
    if _PROGRAM is None:
        _PROGRAM = _build_program()
    return _PROGRAM


def _make_in_maps(x1, x2):
    x2t = np.ascontiguousarray(x2.T)
    in_maps = []
    for c in range(NCORES):
        sl = slice(c * RP, (c + 1) * RP)
        in_maps.append({
            "x1t": np.ascontiguousarray(x1[sl].T),
            "x1n": np.ascontiguousarray(x1[sl]),
            "x2t": x2t,
        })
    return in_maps


def kernel(x1, x2):
    x1 = np.asarray(x1, dtype=np.float32)
    x2 = np.asarray(x2, dtype=np.float32)
    from concourse.bass_utils import run_bass_kernel_spmd

    nc = _get_program()
    res = run_bass_kernel_spmd(nc, _make_in_maps(x1, x2), core_ids=list(range(NCORES)))
    sim = np.concatenate([res.results[c]["simo"] for c in range(NCORES)], axis=0)
    return (sim, np.ascontiguousarray(sim.T))


# revision 16
# speedup vs baseline: 1.1446x; 1.1036x over previous
"""Trainium2 Bass kernel for pairwise Jaccard similarity (nn_ConceptSpace).

Math (per the reference):
    a1 = sigmoid(x1)  [1024, 256]
    a2 = sigmoid(x2)  [1024, 256]
    inter[i, j] = sum_d min(a1[i, d], a2[j, d])
    union[i, j] = s1[i] + s2[j] - inter[i, j]
    out = (sim, sim.T) with sim = inter / union

Sharding: x1 rows are split across the 8 cores (128 rows each); x2 is
replicated.  Each core computes its [128, 1024] slice of `sim`; sim.T is a
free host-side transpose after gathering.

Per-core device algorithm (layout: d on partitions, j on free):
  - a2T units [128 d, 1024 j] (bf16) for d-halves dt=0,1; a1T [128 d, 256] fp32.
  - The [128 i, 1024 j] `inter` block accumulates in PSUM via PE matmuls whose
    stationary operand is a sliding one-hot (column i), reducing each pairwise
    tile over d (partitions) straight into output row i.
  - Pairwise tiles for row i are produced three ways to balance engines:
      * DVE tensor_scalar_min(a2T[dt], a1T[:, i])           -> min tile (4x bf16)
      * ACT activation(Abs, a2T[1], scale=.5, bias=-.5*a1)  -> 0.5|a2-a1| tile,
        accumulated with a NEGATIVE one-hot; the missing 0.5*(s1'+s2') is added
        back into PSUM with two rank-1 matmuls (mask vector x row vector).
      * a fraction of rows folds min0 - h1 on DVE (one TT op) to halve the
        PE streaming cost for that row.
  - epilogue: union = (s1 + s2) - inter;  sim = inter * recip(union)
"""

import sys
from contextlib import ExitStack

for _p in ("/opt/trn_rl_repo", "/root/.axon_site", "/root/.axon_site/_ro/trn_rl_repo",
           "/root/.axon_site/_ro/pypackages"):
    if _p not in sys.path:
        sys.path.insert(0, _p)

import numpy as np

N = 1024          # rows of x1 / x2
D = 256           # feature dim
NCORES = 8
RP = N // NCORES  # rows per core = 128
P = 128           # partitions
JB = 512          # j-block (one PSUM bank of fp32)
NJB = N // JB     # 2 j-blocks

# Row-class split (engine balance):
#   rows [0, ACT_N)              : ACT absdiff + DVE min (+ DVE fold on a
#                                  Bresenham-spread subset of FOLD_N rows)
#   rows [ACT_N, ACT_N+POOL_N)   : DVE min (dt0) + GPSIMD min (dt1)
#   rows [ACT_N+POOL_N, RP)      : DVE min x2
FOLD_N = 54
ACT_N = 72
POOL_N = 56


def _build_program(fold_n=FOLD_N, act_n=ACT_N, pool_n=POOL_N, min_bufs=10, h_bufs=8):
    import concourse.bass as bass
    import concourse.tile as tile
    from concourse import bacc, mybir

    f32 = mybir.dt.float32
    bf16 = mybir.dt.bfloat16
    AF = mybir.ActivationFunctionType

    nc = bacc.Bacc(trn_type="TRN2", debug=False, target_bir_lowering=False)

    x1t = nc.dram_tensor("x1t", [D, RP], f32, kind="ExternalInput")    # x1 slice, transposed
    x1n = nc.dram_tensor("x1n", [RP, D], f32, kind="ExternalInput")    # x1 slice, natural
    x2t = nc.dram_tensor("x2t", [D, N], f32, kind="ExternalInput")     # x2 full, transposed
    simo = nc.dram_tensor("simo", [RP, N], f32, kind="ExternalOutput")

    with ExitStack() as ctx:
        tc = ctx.enter_context(tile.TileContext(nc))
        const = ctx.enter_context(tc.tile_pool(name="const", bufs=1))
        minp = ctx.enter_context(tc.tile_pool(name="minp", bufs=min_bufs))
        hp = ctx.enter_context(tc.tile_pool(name="hp", bufs=h_bufs))
        finp = ctx.enter_context(tc.tile_pool(name="finp", bufs=2))
        psum = ctx.enter_context(
            tc.tile_pool(name="psum", bufs=1, space=bass.MemorySpace.PSUM)
        )

        # ---- load + sigmoid ------------------------------------------------
        X2T = [const.tile([P, N], f32, tag=f"x2t{dt}", name=f"x2t{dt}") for dt in range(2)]
        A2 = [const.tile([P, N], bf16, tag=f"a2{dt}", name=f"a2{dt}") for dt in range(2)]
        for dt in range(2):
            nc.sync.dma_start(X2T[dt][:], x2t[dt * P:(dt + 1) * P, :])
            nc.scalar.activation(A2[dt][:], X2T[dt][:], AF.Sigmoid)

        X1T = const.tile([P, D], f32, tag="x1t", name="x1ts")   # [d_low, (dt, i)]
        A1 = const.tile([P, D], f32, tag="a1", name="a1")
        for dt in range(2):
            hs = slice(dt * RP, (dt + 1) * RP)
            nc.sync.dma_start(X1T[:, hs], x1t[dt * P:(dt + 1) * P, :])
            nc.scalar.activation(A1[:, hs], X1T[:, hs], AF.Sigmoid)
        # negated/halved a1 (dt=1 half) as per-partition bias for ACT absdiff
        A1N = const.tile([P, RP], f32, tag="a1n", name="a1n")
        nc.vector.tensor_scalar_mul(A1N[:], A1[:, RP:D], -0.5)
        # bf16 copy of a1 dt=1 half (for the s1' partition-sum matmul)
        A1B = const.tile([P, RP], bf16, tag="a1b", name="a1b")
        nc.vector.tensor_copy(A1B[:], A1[:, RP:D])

        # s1[i] = sum_d sigmoid(x1[i, d])  (fp32, i on partitions)
        X1N = const.tile([RP, D], f32, tag="x1n", name="x1ns")
        nc.sync.dma_start(X1N[:], x1n[:])
        scr1 = const.tile([RP, D], bf16, tag="scr1", name="scr1")
        s1 = const.tile([RP, 1], f32, tag="s1", name="s1")
        nc.scalar.activation(scr1[:], X1N[:], AF.Sigmoid, accum_out=s1[:])

        # ---- constants -----------------------------------------------------
        # Sliding one-hot buffers: col P-1 is +-1, everything else zero.
        # Bp[:, P-1-i : 2P-1-i] is the one-hot matrix with +1 in col i.
        Bp = const.tile([P, 2 * P - 1], bf16, tag="onehotp", name="onehotp")
        nc.gpsimd.memset(Bp[:], 0.0)
        nc.gpsimd.memset(Bp[:, P - 1:P], 1.0)
        Bn = const.tile([P, 2 * P - 1], bf16, tag="onehotn", name="onehotn")
        nc.gpsimd.memset(Bn[:], 0.0)
        nc.gpsimd.memset(Bn[:, P - 1:P], -1.0)
        onescol = const.tile([P, 1], bf16, tag="onescol", name="onescol")
        nc.gpsimd.memset(onescol[:], 1.0)
        onesrow = const.tile([1, P], f32, tag="onesrow", name="onesrow")
        nc.gpsimd.memset(onesrow[:], 1.0)
        ones512 = const.tile([1, JB], f32, tag="ones512", name="ones512")
        nc.gpsimd.memset(ones512[:], 1.0)
        # ACT-row mask vectors: rows [0, act_n) use the absdiff path
        vmaskf = const.tile([1, P], f32, tag="vmaskf", name="vmaskf")
        nc.gpsimd.memset(vmaskf[:], 0.0)
        nc.gpsimd.memset(vmaskf[:, 0:act_n], 1.0)

        # ---- s2 rows, Sb = s1 + s2 tiles, correction vectors ---------------
        s2row = const.tile([1, N], f32, tag="s2row", name="s2row")
        s2hrow = const.tile([1, N], f32, tag="s2hrow", name="s2hrow")  # 0.5*s2'(dt1)
        Sb = [const.tile([P, JB], f32, tag=f"sb{jb}", name=f"sb{jb}") for jb in range(NJB)]
        for jb in range(NJB):
            js = slice(jb * JB, (jb + 1) * JB)
            s2p = psum.tile([1, JB], f32, tag="s2p", name="s2p")
            for dt in range(2):
                nc.tensor.matmul(
                    s2p[:], onescol[:], A2[dt][:, js],
                    start=(dt == 0), stop=(dt == 1),
                )
            nc.vector.tensor_copy(s2row[:, js], s2p[:])
            s2hp = psum.tile([1, JB], f32, tag="s2hp", name="s2hp")
            nc.tensor.matmul(s2hp[:], onescol[:], A2[1][:, js], start=True, stop=True)
            nc.vector.tensor_scalar_mul(s2hrow[:, js], s2hp[:], 0.5)
            sbp = psum.tile([P, JB], f32, tag="sbp", name="sbp")
            nc.tensor.matmul(sbp[:], onesrow[:], s2row[:, js], start=True, stop=True)
            nc.vector.tensor_scalar_add(Sb[jb][:], sbp[:], s1[:])

        # w1[m] = mask[m] * 0.5 * s1'[m]  (s1' = dt1-half row sums of a1)
        s1hp = psum.tile([1, P], f32, tag="s1hp", name="s1hp")
        nc.tensor.matmul(s1hp[:], onescol[:], A1B[:], start=True, stop=True)
        w1f = const.tile([1, P], f32, tag="w1f", name="w1f")
        nc.vector.tensor_scalar_mul(w1f[:], s1hp[:], 0.5)
        w1 = const.tile([1, P], f32, tag="w1", name="w1")
        nc.vector.tensor_mul(w1[:], w1f[:], vmaskf[:])

        # ---- main loop: accumulate inter rows into PSUM --------------------
        acc = [psum.tile([P, JB], f32, tag=f"acc{jb}", name=f"acc{jb}") for jb in range(NJB)]

        def mm(unit, w, i, jb, first, last):
            nc.tensor.matmul(
                acc[jb][:], w, unit[:, jb * JB:(jb + 1) * JB],
                start=first, stop=last,
            )

        # Emission order interleaves ACT rows (indices [0, act_n), which the
        # mask vectors rely on being contiguous) among the DVE rows, so the
        # in-order PE stream never waits on a burst of slow ACT producers.
        # Fold rows are Bresenham-spread across the ACT rows.
        counts = {"a": act_n, "p": pool_n, "d": RP - act_n - pool_n}
        base = {"a": 0, "p": act_n, "d": act_n + pool_n}
        emitted = {"a": 0, "p": 0, "d": 0}
        order = []
        for e in range(RP):
            c = max(counts, key=lambda k: counts[k] * (e + 1) / RP - emitted[k])
            order.append(base[c] + emitted[c])
            emitted[c] += 1
        assert sorted(order) == list(range(RP))

        for step, i in enumerate(order):
            wp = Bp[:, P - 1 - i:2 * P - 1 - i]
            wn = Bn[:, P - 1 - i:2 * P - 1 - i]
            use_act = i < act_n
            use_fold = use_act and act_n > 0 and (
                (i * fold_n) // act_n != ((i + 1) * fold_n) // act_n
            )
            first = step == 0

            m0 = minp.tile([P, N], bf16, tag="m", name="m")
            nc.vector.tensor_scalar_min(m0[:], A2[0][:], A1[:, i:i + 1])
            if use_act:
                h1 = hp.tile([P, N], bf16, tag="h", name="h")
                nc.scalar.activation(
                    h1[:], A2[1][:], AF.Abs, bias=A1N[:, i:i + 1], scale=0.5
                )
                if use_fold:
                    fu = minp.tile([P, N], bf16, tag="m", name="fold")
                    nc.vector.tensor_sub(fu[:], m0[:], h1[:])
                    for jb in range(NJB):
                        mm(fu, wp, i, jb, first, False)
                else:
                    for jb in range(NJB):
                        mm(m0, wp, i, jb, first, False)
                        mm(h1, wn, i, jb, False, False)
            else:
                m1 = minp.tile([P, N], bf16, tag="m", name="m1")
                if i < act_n + pool_n:
                    nc.gpsimd.tensor_scalar_min(m1[:], A2[1][:], A1[:, RP + i:RP + i + 1])
                else:
                    nc.vector.tensor_scalar_min(m1[:], A2[1][:], A1[:, RP + i:RP + i + 1])
                for jb in range(NJB):
                    mm(m0, wp, i, jb, first, False)
                    mm(m1, wp, i, jb, False, False)

        # rank-1 corrections for ACT rows: + mask[m]*0.5*s2'[j] + w1[m]*1
        for jb in range(NJB):
            js = slice(jb * JB, (jb + 1) * JB)
            nc.tensor.matmul(acc[jb][:], vmaskf[:], s2hrow[:, js], start=False, stop=False)
            nc.tensor.matmul(acc[jb][:], w1[:], ones512[:], start=False, stop=True)

        # ---- epilogue: sim = inter / (Sb - inter) --------------------------
        for jb in range(NJB):
            js = slice(jb * JB, (jb + 1) * JB)
            union = finp.tile([P, JB], f32, tag="union", name="union")
            nc.vector.tensor_sub(union[:], Sb[jb][:], acc[jb][:])
            rcp = finp.tile([P, JB], f32, tag="rcp", name="rcp")
            nc.vector.reciprocal_approx_fast(rcp[:], union[:])
            sims = finp.tile([P, JB], f32, tag="sims", name="sims")
            nc.vector.tensor_mul(sims[:], acc[jb][:], rcp[:])
            nc.sync.dma_start(simo[:, js], sims[:])

    nc.compile()
    return nc


_PROGRAM = None


def _get_program():
    global _PROGRAM
    if _PROGRAM is None:
        _PROGRAM = _build_program()
    return _PROGRAM


def _make_in_maps(x1, x2):
    x2t = np.ascontiguousarray(x2.T)
    in_maps = []
    for c in range(NCORES):
        sl = slice(c * RP, (c + 1) * RP)
        in_maps.append({
            "x1t": np.ascontiguousarray(x1[sl].T),
            "x1n": np.ascontiguousarray(x1[sl]),
            "x2t": x2t,
        })
    return in_maps


def kernel(x1, x2):
    x1 = np.asarray(x1, dtype=np.float32)
    x2 = np.asarray(x2, dtype=np.float32)
    from concourse.bass_utils import run_bass_kernel_spmd

    nc = _get_program()
    res = run_bass_kernel_spmd(nc, _make_in_maps(x1, x2), core_ids=list(range(NCORES)))
    sim = np.concatenate([res.results[c]["simo"] for c in range(NCORES)], axis=0)
    return (sim, np.ascontiguousarray(sim.T))


# revision 21
# speedup vs baseline: 4190.9057x; 3661.3369x over previous
"""Trainium2 Bass kernel for pairwise Jaccard similarity (nn_ConceptSpace).

Math (per the reference):
    a1 = sigmoid(x1)  [1024, 256]
    a2 = sigmoid(x2)  [1024, 256]
    inter[i, j] = sum_d min(a1[i, d], a2[j, d])
    union[i, j] = s1[i] + s2[j] - inter[i, j]
    out = (sim, sim.T) with sim = inter / union

Sharding: x1 rows are split across the 8 cores (128 rows each); x2 is
replicated.  Each core computes its [128, 1024] slice of `sim`; sim.T is a
free host-side transpose after gathering.

Per-core device algorithm (layout: d on partitions, j on free):
  - a2T units [128 d, 1024 j] (bf16) for d-halves dt=0,1; a1T [128 d, 256] fp32.
  - The [128 i, 1024 j] `inter` block accumulates in PSUM via PE matmuls whose
    stationary operand is a sliding one-hot (column i), reducing each pairwise
    tile over d (partitions) straight into output row i.
  - Pairwise tiles for row i are produced on three engines to balance load:
      * DVE tensor_scalar_min(a2T[dt], a1T[:, i])           -> min tile (4x bf16)
      * ACT activation(Abs, a2T[1], scale=.5, bias=-.5*a1)  -> 0.5|a2-a1| tile,
        accumulated with a NEGATIVE one-hot; the missing 0.5*(s1'+s2') is added
        back into PSUM with two rank-1 matmuls (mask vector x row vector)
      * GPSIMD tensor_scalar_min for another slice of rows
    FA rows additionally fold min0 - h1 on DVE (one TT op), halving that
    row's PE streaming cost. Emission order interleaves the classes so the
    in-order PE stream never starves behind one slow producer, and a dummy
    warm-up matmul starts the PE p-state ramp during the DMA preamble.
  - epilogue: union = (s1 + s2) - inter;  sim = inter * recip(union)
"""

import sys
from contextlib import ExitStack

for _p in ("/opt/trn_rl_repo", "/root/.axon_site", "/root/.axon_site/_ro/trn_rl_repo",
           "/root/.axon_site/_ro/pypackages"):
    if _p not in sys.path:
        sys.path.insert(0, _p)

import numpy as np

N = 1024          # rows of x1 / x2
D = 256           # feature dim
NCORES = 8
RP = N // NCORES  # rows per core = 128
P = 128           # partitions
JB = 512          # j-block (one PSUM bank of fp32)
NJB = N // JB     # 2 j-blocks

# Row-class layout (engine balance). Fold rows use one DVE
# scalar_tensor_tensor op that computes the dt1 min AND folds it with the
# dt0 unit, halving that row's PE streaming:
#   [0, FA)          fold, dt0 via ACT absdiff (h0), STT subtract   -> 2 MM
#   [FA, FA+FP)      fold, dt0 via GPSIMD min, STT add              -> 2 MM
#   [FA+FP, NF)      fold, dt0 via DVE min, STT add                 -> 2 MM
#   [NF, NF+NA)      nonfold, dt0 DVE min, dt1 ACT absdiff (h1)     -> 4 MM
#   [NF+NA, RP)      nonfold, dt0 DVE min, dt1 GPSIMD min           -> 4 MM
FA = 54
FP = 0
FD = 0
NA = 18


def _build_program(fa=FA, fp=FP, fd=FD, na=NA, min_bufs=10, h_bufs=8):
    import concourse.bass as bass
    import concourse.tile as tile
    from concourse import bacc, mybir

    f32 = mybir.dt.float32
    bf16 = mybir.dt.bfloat16
    AF = mybir.ActivationFunctionType

    nc = bacc.Bacc(trn_type="TRN2", debug=False, target_bir_lowering=False)

    x1t = nc.dram_tensor("x1t", [D, RP], f32, kind="ExternalInput")    # x1 slice, transposed
    x1n = nc.dram_tensor("x1n", [RP, D], f32, kind="ExternalInput")    # x1 slice, natural
    x2t = nc.dram_tensor("x2t", [D, N], f32, kind="ExternalInput")     # x2 full, transposed
    simo = nc.dram_tensor("simo", [RP, N], f32, kind="ExternalOutput")

    with ExitStack() as ctx:
        tc = ctx.enter_context(tile.TileContext(nc))
        const = ctx.enter_context(tc.tile_pool(name="const", bufs=1))
        minp = ctx.enter_context(tc.tile_pool(name="minp", bufs=min_bufs))
        hp = ctx.enter_context(tc.tile_pool(name="hp", bufs=h_bufs))
        finp = ctx.enter_context(tc.tile_pool(name="finp", bufs=2))
        psum = ctx.enter_context(
            tc.tile_pool(name="psum", bufs=1, space=bass.MemorySpace.PSUM)
        )

        # ---- load + sigmoid ------------------------------------------------
        X2T = [const.tile([P, N], f32, tag=f"x2t{dt}", name=f"x2t{dt}") for dt in range(2)]
        A2 = [const.tile([P, N], bf16, tag=f"a2{dt}", name=f"a2{dt}") for dt in range(2)]
        for dt in range(2):
            nc.sync.dma_start(X2T[dt][:], x2t[dt * P:(dt + 1) * P, :])
            nc.scalar.activation(A2[dt][:], X2T[dt][:], AF.Sigmoid)

        X1T = const.tile([P, D], f32, tag="x1t", name="x1ts")   # [d_low, (dt, i)]
        A1 = const.tile([P, D], f32, tag="a1", name="a1")
        for dt in range(2):
            hs = slice(dt * RP, (dt + 1) * RP)
            nc.sync.dma_start(X1T[:, hs], x1t[dt * P:(dt + 1) * P, :])
            nc.scalar.activation(A1[:, hs], X1T[:, hs], AF.Sigmoid)
        # negated/halved a1 (dt=1 half) as per-partition bias for ACT absdiff
        A1N = const.tile([P, RP], f32, tag="a1n", name="a1n")
        nc.vector.tensor_scalar_mul(A1N[:], A1[:, RP:D], -0.5)
        # bf16 copy of a1 (for the s1-half partition-sum matmuls)
        A1B = const.tile([P, D], bf16, tag="a1b", name="a1b")
        nc.vector.tensor_copy(A1B[:], A1[:])

        # s1[i] = sum_d sigmoid(x1[i, d])  (fp32, i on partitions)
        X1N = const.tile([RP, D], f32, tag="x1n", name="x1ns")
        nc.sync.dma_start(X1N[:], x1n[:])
        scr1 = const.tile([RP, D], bf16, tag="scr1", name="scr1")
        s1 = const.tile([RP, 1], f32, tag="s1", name="s1")
        nc.scalar.activation(scr1[:], X1N[:], AF.Sigmoid, accum_out=s1[:])

        # ---- constants -----------------------------------------------------
        # Sliding one-hot buffers: col P-1 is +-1, everything else zero.
        # Bp[:, P-1-i : 2P-1-i] is the one-hot matrix with +1 in col i.
        Bp = const.tile([P, 2 * P - 1], bf16, tag="onehotp", name="onehotp")
        nc.gpsimd.memset(Bp[:], 0.0)
        nc.gpsimd.memset(Bp[:, P - 1:P], 1.0)
        Bn = const.tile([P, 2 * P - 1], bf16, tag="onehotn", name="onehotn")
        nc.gpsimd.memset(Bn[:], 0.0)
        nc.gpsimd.memset(Bn[:, P - 1:P], -1.0)
        onescol = const.tile([P, 1], bf16, tag="onescol", name="onescol")
        nc.gpsimd.memset(onescol[:], 1.0)
        onesrow = const.tile([1, P], f32, tag="onesrow", name="onesrow")
        nc.gpsimd.memset(onesrow[:], 1.0)
        ones512 = const.tile([1, JB], f32, tag="ones512", name="ones512")
        nc.gpsimd.memset(ones512[:], 1.0)
        # masks for rows whose dt0 (rows [0, fa)) / dt1 (rows [nf, nf+na))
        # pair tiles come from the ACT absdiff path
        nf = fa + fp + fd
        vmask1 = const.tile([1, P], f32, tag="vmask1", name="vmask1")
        nc.gpsimd.memset(vmask1[:], 0.0)
        if fa:
            nc.gpsimd.memset(vmask1[:, 0:fa], 1.0)
        if na:
            nc.gpsimd.memset(vmask1[:, nf:nf + na], 1.0)

        # ---- warm up the PE p-state ramp before the real stream ------------
        warmt = const.tile([P, JB], bf16, tag="warmt", name="warmt")
        nc.gpsimd.memset(warmt[:], 0.0)
        wpsum = psum.tile([1, JB], f32, tag="wpsum", name="wpsum")
        nc.tensor.matmul(wpsum[:], onescol[:], warmt[:], start=True, stop=True)

        # ---- s2 rows, Sb = s1 + s2 tiles, correction vectors ---------------
        # s2half[h][j] = 0.5 * sum_{d in half h} a2[d, j]
        s2row = const.tile([1, N], f32, tag="s2row", name="s2row")
        s2half1 = const.tile([1, N], f32, tag="s2h1", name="s2h1")
        Sb = [const.tile([P, JB], f32, tag=f"sb{jb}", name=f"sb{jb}") for jb in range(NJB)]
        for jb in range(NJB):
            js = slice(jb * JB, (jb + 1) * JB)
            s2p = psum.tile([1, JB], f32, tag="s2p", name="s2p")
            for dt in range(2):
                nc.tensor.matmul(
                    s2p[:], onescol[:], A2[dt][:, js],
                    start=(dt == 0), stop=(dt == 1),
                )
            nc.vector.tensor_copy(s2row[:, js], s2p[:])
            s2hp = psum.tile([1, JB], f32, tag="s2hp", name="s2hp")
            nc.tensor.matmul(s2hp[:], onescol[:], A2[1][:, js], start=True, stop=True)
            nc.vector.tensor_scalar_mul(s2half1[:, js], s2hp[:], 0.5)
            sbp = psum.tile([P, JB], f32, tag="sbp", name="sbp")
            nc.tensor.matmul(sbp[:], onesrow[:], s2row[:, js], start=True, stop=True)
            nc.vector.tensor_scalar_add(Sb[jb][:], sbp[:], s1[:])

        # w1[m] = mask1[m] * 0.5 * s1half1[m]
        s1hp = psum.tile([1, P], f32, tag="s1hp", name="s1hp")
        nc.tensor.matmul(s1hp[:], onescol[:], A1B[:, RP:D], start=True, stop=True)
        w1f = const.tile([1, P], f32, tag="w1f", name="w1f")
        nc.vector.tensor_scalar_mul(w1f[:], s1hp[:], 0.5)
        w1 = const.tile([1, P], f32, tag="w1", name="w1")
        nc.vector.tensor_mul(w1[:], w1f[:], vmask1[:])

        # ---- main loop: accumulate inter rows into PSUM --------------------
        acc = [psum.tile([P, JB], f32, tag=f"acc{jb}", name=f"acc{jb}") for jb in range(NJB)]

        def mm(unit, w, i, jb, first, last):
            nc.tensor.matmul(
                acc[jb][:], w, unit[:, jb * JB:(jb + 1) * JB],
                start=first, stop=last,
            )

        # Emission order interleaves the row classes so every engine's feed
        # stays steady; the in-order PE stream never waits on a burst of one
        # slow producer. PSUM accumulation is order-independent.
        classes = [("fa", 0, fa), ("fp", fa, fp), ("fd", fa + fp, fd),
                   ("na", nf, na), ("np", nf + na, RP - nf - na)]
        counts = {c: n for c, _, n in classes}
        base = {c: b for c, b, _ in classes}
        emitted = {c: 0 for c, _, _ in classes}
        order = []
        for e in range(RP):
            c = max(counts, key=lambda k: counts[k] * (e + 1) / RP - emitted[k])
            order.append((c, base[c] + emitted[c]))
            emitted[c] += 1
        assert sorted(i for _, i in order) == list(range(RP))

        Alu = mybir.AluOpType
        for step, (cls, i) in enumerate(order):
            wp = Bp[:, P - 1 - i:2 * P - 1 - i]
            wn = Bn[:, P - 1 - i:2 * P - 1 - i]
            first = step == 0

            if cls in ("fa", "fp", "fd"):
                # fold row: produce both halves, fold on DVE -> 2 matmuls.
                m0 = minp.tile([P, N], bf16, tag="m", name="m0f")
                nc.vector.tensor_scalar_min(m0[:], A2[0][:], A1[:, i:i + 1])
                if cls == "fa":
                    # dt1 via ACT absdiff; fu = m0 - h1 (corrections add back
                    # the 0.5*(s1'+s2') of half 1)
                    u1 = hp.tile([P, N], bf16, tag="h", name="h1f")
                    nc.scalar.activation(
                        u1[:], A2[1][:], AF.Abs, bias=A1N[:, i:i + 1], scale=0.5
                    )
                    fu = minp.tile([P, N], bf16, tag="m", name="fold")
                    nc.vector.tensor_sub(fu[:], m0[:], u1[:])
                else:
                    if cls == "fp":
                        u1 = minp.tile([P, N], bf16, tag="m", name="mp1")
                        nc.gpsimd.tensor_scalar_min(u1[:], A2[1][:], A1[:, RP + i:RP + i + 1])
                    else:
                        u1 = minp.tile([P, N], bf16, tag="m", name="md1")
                        nc.vector.tensor_scalar_min(u1[:], A2[1][:], A1[:, RP + i:RP + i + 1])
                    fu = minp.tile([P, N], bf16, tag="m", name="fold")
                    nc.vector.tensor_add(fu[:], m0[:], u1[:])
                for jb in range(NJB):
                    mm(fu, wp, i, jb, first, False)
            elif cls == "na":
                m0 = minp.tile([P, N], bf16, tag="m", name="m")
                nc.vector.tensor_scalar_min(m0[:], A2[0][:], A1[:, i:i + 1])
                h1 = hp.tile([P, N], bf16, tag="h", name="h")
                nc.scalar.activation(
                    h1[:], A2[1][:], AF.Abs, bias=A1N[:, i:i + 1], scale=0.5
                )
                for jb in range(NJB):
                    mm(m0, wp, i, jb, first, False)
                    mm(h1, wn, i, jb, False, False)
            else:
                m0 = minp.tile([P, N], bf16, tag="m", name="m")
                nc.vector.tensor_scalar_min(m0[:], A2[0][:], A1[:, i:i + 1])
                m1 = minp.tile([P, N], bf16, tag="m", name="m1")
                nc.gpsimd.tensor_scalar_min(m1[:], A2[1][:], A1[:, RP + i:RP + i + 1])
                for jb in range(NJB):
                    mm(m0, wp, i, jb, first, False)
                    mm(m1, wp, i, jb, False, False)

        # rank-1 corrections for ACT-absdiff rows (all on half 1):
        #   acc += mask1[m] * 0.5*s2half1[j]  +  w1[m] * 1
        for jb in range(NJB):
            js = slice(jb * JB, (jb + 1) * JB)
            nc.tensor.matmul(acc[jb][:], vmask1[:], s2half1[:, js],
                             start=False, stop=False)
            nc.tensor.matmul(acc[jb][:], w1[:], ones512[:], start=False, stop=True)

        # ---- epilogue: sim = inter / (Sb - inter) --------------------------
        for jb in range(NJB):
            js = slice(jb * JB, (jb + 1) * JB)
            union = finp.tile([P, JB], f32, tag="union", name="union")
            nc.vector.tensor_sub(union[:], Sb[jb][:], acc[jb][:])
            rcp = finp.tile([P, JB], f32, tag="rcp", name="rcp")
            nc.vector.reciprocal_approx_fast(rcp[:], union[:])
            sims = finp.tile([P, JB], f32, tag="sims", name="sims")
            nc.vector.tensor_mul(sims[:], acc[jb][:], rcp[:])
            nc.sync.dma_start(simo[:, js], sims[:])

    nc.compile()
    return nc


_PROGRAM = None


def _get_program():
    global _PROGRAM
    if _PROGRAM is None:
        _PROGRAM = _build_program()
    return _PROGRAM


def _make_in_maps(x1, x2):
    x2t = np.ascontiguousarray(x2.T)
    in_maps = []
    for c in range(NCORES):
        sl = slice(c * RP, (c + 1) * RP)
        in_maps.append({
            "x1t": np.ascontiguousarray(x1[sl].T),
            "x1n": np.ascontiguousarray(x1[sl]),
            "x2t": x2t,
        })
    return in_maps


def kernel(x1, x2):
    x1 = np.asarray(x1, dtype=np.float32)
    x2 = np.asarray(x2, dtype=np.float32)
    from concourse.bass_utils import run_bass_kernel_spmd

    nc = _get_program()
    res = run_bass_kernel_spmd(nc, _make_in_maps(x1, x2), core_ids=list(range(NCORES)))
    sim = np.concatenate([res.results[c]["simo"] for c in range(NCORES)], axis=0)
    return (sim, np.ascontiguousarray(sim.T))


# revision 22
# speedup vs baseline: 4215.9465x; 1.0060x over previous
"""Trainium2 Bass kernel for pairwise Jaccard similarity (nn_ConceptSpace).

Math (per the reference):
    a1 = sigmoid(x1)  [1024, 256]
    a2 = sigmoid(x2)  [1024, 256]
    inter[i, j] = sum_d min(a1[i, d], a2[j, d])
    union[i, j] = s1[i] + s2[j] - inter[i, j]
    out = (sim, sim.T) with sim = inter / union

Sharding: x1 rows are split across the 8 cores (128 rows each); x2 is
replicated.  Each core computes its [128, 1024] slice of `sim`; sim.T is a
free host-side transpose after gathering.

Per-core device algorithm (layout: d on partitions, j on free):
  - a2T units [128 d, 1024 j] (bf16) for d-halves dt=0,1; a1T [128 d, 256] fp32.
  - The [128 i, 1024 j] `inter` block accumulates in PSUM via PE matmuls whose
    stationary operand is a sliding one-hot (column i), reducing each pairwise
    tile over d (partitions) straight into output row i.
  - Pairwise tiles for row i are produced on three engines to balance load:
      * DVE tensor_scalar_min(a2T[dt], a1T[:, i])           -> min tile (4x bf16)
      * ACT activation(Abs, a2T[1], scale=.5, bias=-.5*a1)  -> 0.5|a2-a1| tile,
        accumulated with a NEGATIVE one-hot; the missing 0.5*(s1'+s2') is added
        back into PSUM with two rank-1 matmuls (mask vector x row vector)
      * GPSIMD tensor_scalar_min for another slice of rows
    FA rows additionally fold min0 - h1 on DVE (one TT op), halving that
    row's PE streaming cost. Emission order interleaves the classes so the
    in-order PE stream never starves behind one slow producer, and a dummy
    warm-up matmul starts the PE p-state ramp during the DMA preamble.
  - epilogue: union = (s1 + s2) - inter;  sim = inter * recip(union)
"""

import sys
from contextlib import ExitStack

for _p in ("/opt/trn_rl_repo", "/root/.axon_site", "/root/.axon_site/_ro/trn_rl_repo",
           "/root/.axon_site/_ro/pypackages"):
    if _p not in sys.path:
        sys.path.insert(0, _p)

import numpy as np

N = 1024          # rows of x1 / x2
D = 256           # feature dim
NCORES = 8
RP = N // NCORES  # rows per core = 128
P = 128           # partitions
JB = 512          # j-block (one PSUM bank of fp32)
NJB = N // JB     # 2 j-blocks

# Row-class layout (engine balance). Fold rows use one DVE
# scalar_tensor_tensor op that computes the dt1 min AND folds it with the
# dt0 unit, halving that row's PE streaming:
#   [0, FA)          fold, dt0 via ACT absdiff (h0), STT subtract   -> 2 MM
#   [FA, FA+FP)      fold, dt0 via GPSIMD min, STT add              -> 2 MM
#   [FA+FP, NF)      fold, dt0 via DVE min, STT add                 -> 2 MM
#   [NF, NF+NA)      nonfold, dt0 DVE min, dt1 ACT absdiff (h1)     -> 4 MM
#   [NF+NA, RP)      nonfold, dt0 DVE min, dt1 GPSIMD min           -> 4 MM
FA = 54
FP = 0
FD = 0
NA = 18


def _build_program(fa=FA, fp=FP, fd=FD, na=NA, min_bufs=10, h_bufs=8):
    import concourse.bass as bass
    import concourse.tile as tile
    from concourse import bacc, mybir

    f32 = mybir.dt.float32
    bf16 = mybir.dt.bfloat16
    AF = mybir.ActivationFunctionType

    nc = bacc.Bacc(trn_type="TRN2", debug=False, target_bir_lowering=False)

    x1t = nc.dram_tensor("x1t", [D, RP], f32, kind="ExternalInput")    # x1 slice, transposed
    x1n = nc.dram_tensor("x1n", [RP, D], f32, kind="ExternalInput")    # x1 slice, natural
    x2t = nc.dram_tensor("x2t", [D, N], f32, kind="ExternalInput")     # x2 full, transposed
    simo = nc.dram_tensor("simo", [RP, N], f32, kind="ExternalOutput")

    with ExitStack() as ctx:
        tc = ctx.enter_context(tile.TileContext(nc))
        const = ctx.enter_context(tc.tile_pool(name="const", bufs=1))
        minp = ctx.enter_context(tc.tile_pool(name="minp", bufs=min_bufs))
        hp = ctx.enter_context(tc.tile_pool(name="hp", bufs=h_bufs))
        finp = ctx.enter_context(tc.tile_pool(name="finp", bufs=2))
        psum = ctx.enter_context(
            tc.tile_pool(name="psum", bufs=1, space=bass.MemorySpace.PSUM)
        )

        # ---- load + sigmoid ------------------------------------------------
        X2T = [const.tile([P, N], f32, tag=f"x2t{dt}", name=f"x2t{dt}") for dt in range(2)]
        A2 = [const.tile([P, N], bf16, tag=f"a2{dt}", name=f"a2{dt}") for dt in range(2)]
        for dt in range(2):
            nc.sync.dma_start(X2T[dt][:], x2t[dt * P:(dt + 1) * P, :])
            nc.scalar.activation(A2[dt][:], X2T[dt][:], AF.Sigmoid)

        X1T = const.tile([P, D], f32, tag="x1t", name="x1ts")   # [d_low, (dt, i)]
        A1 = const.tile([P, D], f32, tag="a1", name="a1")
        for dt in range(2):
            hs = slice(dt * RP, (dt + 1) * RP)
            nc.sync.dma_start(X1T[:, hs], x1t[dt * P:(dt + 1) * P, :])
            nc.scalar.activation(A1[:, hs], X1T[:, hs], AF.Sigmoid)
        # negated/halved a1 (dt=1 half) as per-partition bias for ACT absdiff
        A1N = const.tile([P, RP], f32, tag="a1n", name="a1n")
        nc.vector.tensor_scalar_mul(A1N[:], A1[:, RP:D], -0.5)
        # bf16 copy of a1 (for the s1-half partition-sum matmuls)
        A1B = const.tile([P, D], bf16, tag="a1b", name="a1b")
        nc.vector.tensor_copy(A1B[:], A1[:])

        # s1[i] = sum_d sigmoid(x1[i, d])  (fp32, i on partitions)
        X1N = const.tile([RP, D], f32, tag="x1n", name="x1ns")
        nc.sync.dma_start(X1N[:], x1n[:])
        scr1 = const.tile([RP, D], bf16, tag="scr1", name="scr1")
        s1 = const.tile([RP, 1], f32, tag="s1", name="s1")
        nc.scalar.activation(scr1[:], X1N[:], AF.Sigmoid, accum_out=s1[:])

        # ---- constants -----------------------------------------------------
        # Sliding one-hot buffers: col P-1 is +-1, everything else zero.
        # Bp[:, P-1-i : 2P-1-i] is the one-hot matrix with +1 in col i.
        Bp = const.tile([P, 2 * P - 1], bf16, tag="onehotp", name="onehotp")
        nc.gpsimd.memset(Bp[:], 0.0)
        nc.gpsimd.memset(Bp[:, P - 1:P], 1.0)
        Bn = const.tile([P, 2 * P - 1], bf16, tag="onehotn", name="onehotn")
        nc.gpsimd.memset(Bn[:], 0.0)
        nc.gpsimd.memset(Bn[:, P - 1:P], -1.0)
        onescol = const.tile([P, 1], bf16, tag="onescol", name="onescol")
        nc.gpsimd.memset(onescol[:], 1.0)
        onesrow = const.tile([1, P], f32, tag="onesrow", name="onesrow")
        nc.gpsimd.memset(onesrow[:], 1.0)
        ones512 = const.tile([1, JB], f32, tag="ones512", name="ones512")
        nc.gpsimd.memset(ones512[:], 1.0)
        # masks for rows whose dt0 (rows [0, fa)) / dt1 (rows [nf, nf+na))
        # pair tiles come from the ACT absdiff path
        nf = fa + fp + fd
        vmask1 = const.tile([1, P], f32, tag="vmask1", name="vmask1")
        nc.gpsimd.memset(vmask1[:], 0.0)
        if fa:
            nc.gpsimd.memset(vmask1[:, 0:fa], 1.0)
        if na:
            nc.gpsimd.memset(vmask1[:, nf:nf + na], 1.0)

        # ---- warm up the PE p-state ramp before the real stream ------------
        warmt = const.tile([P, JB], bf16, tag="warmt", name="warmt")
        nc.gpsimd.memset(warmt[:], 0.0)
        wpsum = psum.tile([1, JB], f32, tag="wpsum", name="wpsum")
        nc.tensor.matmul(wpsum[:], onescol[:], warmt[:], start=True, stop=True)

        # ---- s2 rows, Sb = s1 + s2 tiles, correction vectors ---------------
        # s2half[h][j] = 0.5 * sum_{d in half h} a2[d, j]
        s2row = const.tile([1, N], f32, tag="s2row", name="s2row")
        s2half1 = const.tile([1, N], f32, tag="s2h1", name="s2h1")
        Sb = [const.tile([P, JB], f32, tag=f"sb{jb}", name=f"sb{jb}") for jb in range(NJB)]
        for jb in range(NJB):
            js = slice(jb * JB, (jb + 1) * JB)
            s2p = psum.tile([1, JB], f32, tag="s2p", name="s2p")
            for dt in range(2):
                nc.tensor.matmul(
                    s2p[:], onescol[:], A2[dt][:, js],
                    start=(dt == 0), stop=(dt == 1),
                )
            nc.vector.tensor_copy(s2row[:, js], s2p[:])
            s2hp = psum.tile([1, JB], f32, tag="s2hp", name="s2hp")
            nc.tensor.matmul(s2hp[:], onescol[:], A2[1][:, js], start=True, stop=True)
            nc.vector.tensor_scalar_mul(s2half1[:, js], s2hp[:], 0.5)
            sbp = psum.tile([P, JB], f32, tag="sbp", name="sbp")
            nc.tensor.matmul(sbp[:], onesrow[:], s2row[:, js], start=True, stop=True)
            nc.scalar.activation(Sb[jb][:], sbp[:], AF.Identity, bias=s1[:])

        # w1[m] = mask1[m] * 0.5 * s1half1[m]
        s1hp = psum.tile([1, P], f32, tag="s1hp", name="s1hp")
        nc.tensor.matmul(s1hp[:], onescol[:], A1B[:, RP:D], start=True, stop=True)
        w1f = const.tile([1, P], f32, tag="w1f", name="w1f")
        nc.vector.tensor_scalar_mul(w1f[:], s1hp[:], 0.5)
        w1 = const.tile([1, P], f32, tag="w1", name="w1")
        nc.vector.tensor_mul(w1[:], w1f[:], vmask1[:])

        # ---- main loop: accumulate inter rows into PSUM --------------------
        acc = [psum.tile([P, JB], f32, tag=f"acc{jb}", name=f"acc{jb}") for jb in range(NJB)]

        def mm(unit, w, i, jb, first, last):
            nc.tensor.matmul(
                acc[jb][:], w, unit[:, jb * JB:(jb + 1) * JB],
                start=first, stop=last,
            )

        # Emission order interleaves the row classes so every engine's feed
        # stays steady; the in-order PE stream never waits on a burst of one
        # slow producer. PSUM accumulation is order-independent.
        classes = [("fa", 0, fa), ("fp", fa, fp), ("fd", fa + fp, fd),
                   ("na", nf, na), ("np", nf + na, RP - nf - na)]
        counts = {c: n for c, _, n in classes}
        base = {c: b for c, b, _ in classes}
        emitted = {c: 0 for c, _, _ in classes}
        order = []
        for e in range(RP):
            c = max(counts, key=lambda k: counts[k] * (e + 1) / RP - emitted[k])
            order.append((c, base[c] + emitted[c]))
            emitted[c] += 1
        assert sorted(i for _, i in order) == list(range(RP))

        Alu = mybir.AluOpType
        for step, (cls, i) in enumerate(order):
            wp = Bp[:, P - 1 - i:2 * P - 1 - i]
            wn = Bn[:, P - 1 - i:2 * P - 1 - i]
            first = step == 0

            if cls in ("fa", "fp", "fd"):
                # fold row: produce both halves, fold on DVE -> 2 matmuls.
                m0 = minp.tile([P, N], bf16, tag="m", name="m0f")
                nc.vector.tensor_scalar_min(m0[:], A2[0][:], A1[:, i:i + 1])
                if cls == "fa":
                    # dt1 via ACT absdiff; fu = m0 - h1 (corrections add back
                    # the 0.5*(s1'+s2') of half 1)
                    u1 = hp.tile([P, N], bf16, tag="h", name="h1f")
                    nc.scalar.activation(
                        u1[:], A2[1][:], AF.Abs, bias=A1N[:, i:i + 1], scale=0.5
                    )
                    fu = minp.tile([P, N], bf16, tag="m", name="fold")
                    nc.vector.tensor_sub(fu[:], m0[:], u1[:])
                else:
                    if cls == "fp":
                        u1 = minp.tile([P, N], bf16, tag="m", name="mp1")
                        nc.gpsimd.tensor_scalar_min(u1[:], A2[1][:], A1[:, RP + i:RP + i + 1])
                    else:
                        u1 = minp.tile([P, N], bf16, tag="m", name="md1")
                        nc.vector.tensor_scalar_min(u1[:], A2[1][:], A1[:, RP + i:RP + i + 1])
                    fu = minp.tile([P, N], bf16, tag="m", name="fold")
                    nc.vector.tensor_add(fu[:], m0[:], u1[:])
                for jb in range(NJB):
                    mm(fu, wp, i, jb, first, False)
            elif cls == "na":
                m0 = minp.tile([P, N], bf16, tag="m", name="m")
                nc.vector.tensor_scalar_min(m0[:], A2[0][:], A1[:, i:i + 1])
                h1 = hp.tile([P, N], bf16, tag="h", name="h")
                nc.scalar.activation(
                    h1[:], A2[1][:], AF.Abs, bias=A1N[:, i:i + 1], scale=0.5
                )
                for jb in range(NJB):
                    mm(m0, wp, i, jb, first, False)
                    mm(h1, wn, i, jb, False, False)
            else:
                m0 = minp.tile([P, N], bf16, tag="m", name="m")
                nc.vector.tensor_scalar_min(m0[:], A2[0][:], A1[:, i:i + 1])
                m1 = minp.tile([P, N], bf16, tag="m", name="m1")
                nc.gpsimd.tensor_scalar_min(m1[:], A2[1][:], A1[:, RP + i:RP + i + 1])
                for jb in range(NJB):
                    mm(m0, wp, i, jb, first, False)
                    mm(m1, wp, i, jb, False, False)

        # rank-1 corrections for ACT-absdiff rows (all on half 1):
        #   acc += mask1[m] * 0.5*s2half1[j]  +  w1[m] * 1
        for jb in range(NJB):
            js = slice(jb * JB, (jb + 1) * JB)
            nc.tensor.matmul(acc[jb][:], vmask1[:], s2half1[:, js],
                             start=False, stop=False)
            nc.tensor.matmul(acc[jb][:], w1[:], ones512[:], start=False, stop=True)

        # ---- epilogue: sim = inter / (Sb - inter) --------------------------
        for jb in range(NJB):
            js = slice(jb * JB, (jb + 1) * JB)
            union = finp.tile([P, JB], f32, tag="union", name="union")
            nc.vector.tensor_sub(union[:], Sb[jb][:], acc[jb][:])
            rcp = finp.tile([P, JB], f32, tag="rcp", name="rcp")
            nc.vector.reciprocal_approx_fast(rcp[:], union[:])
            sims = finp.tile([P, JB], f32, tag="sims", name="sims")
            nc.vector.tensor_mul(sims[:], acc[jb][:], rcp[:])
            nc.sync.dma_start(simo[:, js], sims[:])

    nc.compile()
    return nc


_PROGRAM = None


def _get_program():
    global _PROGRAM
    if _PROGRAM is None:
        _PROGRAM = _build_program()
    return _PROGRAM


def _make_in_maps(x1, x2):
    x2t = np.ascontiguousarray(x2.T)
    in_maps = []
    for c in range(NCORES):
        sl = slice(c * RP, (c + 1) * RP)
        in_maps.append({
            "x1t": np.ascontiguousarray(x1[sl].T),
            "x1n": np.ascontiguousarray(x1[sl]),
            "x2t": x2t,
        })
    return in_maps


def kernel(x1, x2):
    x1 = np.asarray(x1, dtype=np.float32)
    x2 = np.asarray(x2, dtype=np.float32)
    from concourse.bass_utils import run_bass_kernel_spmd

    nc = _get_program()
    res = run_bass_kernel_spmd(nc, _make_in_maps(x1, x2), core_ids=list(range(NCORES)))
    sim = np.concatenate([res.results[c]["simo"] for c in range(NCORES)], axis=0)
    return (sim, np.ascontiguousarray(sim.T))
